# revision 7
# baseline (speedup 1.0000x reference)
"""GRANMixtureBernoulli loss kernel for 8 TRN2 NeuronCores (Bass/Tile).

Strategy (per sharding hint): group each subgraph's edges on one device.
The host sorts edges by subgraph, pads every subgraph to a uniform edge
count L, and shards 512 subgraphs per core.  Per-core staged layout puts
subgraphs on SBUF partitions and edge positions on the free dimension, so
the segment sums are plain free-dim reductions:

  red_adj[s,k] = sum_pos softplus(lt) - sum_pos lt*label
  red_la[s,k]  = sum_pos la

On device (per core, per (k, seg-chunk) [128, L] tile):
  - ScalarE: u = Exp(lt); sp = Ln(u + 1) with accum_out -> sum softplus
    (no softplus ACT table exists in this toolchain; Exp+Ln share one set)
  - VectorE: tensor_tensor_reduce(mult, add) -> sum lt*label in one pass
  - VectorE: tensor_reduce(add) -> sum la
Pad values (lt=-60, label=0, la=0) make padded positions contribute 0.
A small per-chunk epilogue (log_softmax over K, logsumexp over K) runs on
DVE/ACT, the 128-partition sum of log_prob uses a ones-vector matmul, and
each core DMAs one scalar partial; the host sums 8 partials into the loss.
"""

import numpy as np
from contextlib import ExitStack

import concourse.bass as bass
import concourse.tile as tile
from concourse import mybir
from concourse.vector_clock import ScopedClock
from concourse.bass_utils import run_bass_kernel_spmd

E = 4194304
K = 20
S = 4096
N_CORES = 8
SEG_PER_CORE = S // N_CORES   # 512
SC = SEG_PER_CORE // 128      # 4 seg-chunks of 128 partitions

F32 = mybir.dt.float32
AF = mybir.ActivationFunctionType
ALU = mybir.AluOpType
AX = mybir.AxisListType

LT_PAD = -60.0  # exp(-60) == 0 in f32 after ln(1+u); softplus(pad) == 0


SplitDrainTileContext = tile.TileContext


def split_multi_waits(nc):
    """This walrus build accepts at most ONE sem wait per instruction.
    Hoist extra waits onto injected same-engine NoOps placed just before
    the instruction (waits execute on the issuing engine's sequencer, so
    ordering is preserved)."""
    n = 0
    for fn in nc.m.functions:
        for blk in fn.blocks:
            new = []
            changed = False
            for inst in blk.instructions:
                si = inst.sync_info
                waits = list(si.on_wait) if si and si.on_wait else []
                if len(waits) > 1:
                    changed = True
                    for w in waits[:-1]:
                        nop = mybir.InstNoOp(name=f"splitw-{n}")
                        n += 1
                        nop.engine = inst.engine
                        nop.sync_info = mybir.SyncInfo(on_wait=[w], on_update=[])
                        new.append(nop)
                    inst.sync_info = mybir.SyncInfo(
                        on_wait=[waits[-1]], on_update=list(si.on_update or []))
                new.append(inst)
            if changed:
                blk.instructions = new


def build_graph(L, reps=1):
    """Build the per-core SPMD graph for padded segment length L.

    reps > 1 unrolls the whole computation reps times (identical work,
    outputs overwritten) -- used by the test harness to measure per-
    iteration HW time as a slope, cancelling fixed dispatch overheads.
    """
    nc = bass.Bass()
    lt_ext = nc.declare_dram_parameter("lt", [K, SC, 128, L], F32, isOutput=False)
    la_ext = nc.declare_dram_parameter("la", [K, SC, 128, L], F32, isOutput=False)
    lb_ext = nc.declare_dram_parameter("lb", [SC, 128, L], F32, isOutput=False)
    cnt_ext = nc.declare_dram_parameter("cnt", [128, SC], F32, isOutput=False)
    out_ext = nc.declare_dram_parameter("out", [1, 1], F32, isOutput=True)

    with SplitDrainTileContext(nc) as tc, ExitStack() as ctx:
        const_p = ctx.enter_context(tc.tile_pool(name="const", bufs=1))
        lt_p = ctx.enter_context(tc.tile_pool(name="lt", bufs=3))
        la_p = ctx.enter_context(tc.tile_pool(name="la", bufs=3))
        u_p = ctx.enter_context(tc.tile_pool(name="u", bufs=2))
        sp_p = ctx.enter_context(tc.tile_pool(name="sp", bufs=2))
        pr_p = ctx.enter_context(tc.tile_pool(name="pr", bufs=2))
        acc_p = ctx.enter_context(tc.tile_pool(name="acc", bufs=2))
        epi_p = ctx.enter_context(tc.tile_pool(name="epi", bufs=2))
        ps_p = ctx.enter_context(tc.tile_pool(name="ps", bufs=1, space="PSUM"))

        # Resident tiles: labels per seg-chunk, counts, ones vector.
        lb_t = []
        for sc in range(SC):
            t = const_p.tile([128, L], F32, tag=f"lb{sc}", name=f"lb{sc}")
            nc.sync.dma_start(t[:], lb_ext[sc])
            lb_t.append(t)
        cnt_t = const_p.tile([128, SC], F32, tag="cnt")
        nc.sync.dma_start(cnt_t[:], cnt_ext[:])
        ones_t = const_p.tile([128, 1], F32, tag="ones")
        nc.vector.memset(ones_t[:], 1.0)

        for _ in range(reps):
            a1 = [acc_p.tile([128, K], F32, tag=f"a1_{sc}", name=f"a1_{sc}")
                  for sc in range(SC)]
            a2 = [acc_p.tile([128, K], F32, tag=f"a2_{sc}", name=f"a2_{sc}")
                  for sc in range(SC)]
            lac = [acc_p.tile([128, K], F32, tag=f"lac_{sc}", name=f"lac_{sc}")
                   for sc in range(SC)]

            for k in range(K):
                for sc in range(SC):
                    lt_t = lt_p.tile([128, L], F32)
                    nc.sync.dma_start(lt_t[:], lt_ext[k, sc])
                    la_t = la_p.tile([128, L], F32)
                    nc.sync.dma_start(la_t[:], la_ext[k, sc])

                    u_t = u_p.tile([128, L], F32)
                    nc.scalar.activation(u_t[:], lt_t[:], AF.Exp)
                    sp_t = sp_p.tile([128, L], F32)
                    nc.scalar.activation(sp_t[:], u_t[:], AF.Ln, bias=1.0,
                                         accum_out=a1[sc][:, k:k + 1])

                    pr_t = pr_p.tile([128, L], F32)
                    nc.vector.tensor_mul(pr_t[:], lt_t[:], lb_t[sc][:])
                    nc.vector.tensor_reduce(
                        out=a2[sc][:, k:k + 1], in_=pr_t[:],
                        axis=AX.X, op=ALU.add)

                    lac_t = sp_p.tile([128, L], F32, tag="lacopy", name="lacopy")
                    nc.scalar.activation(lac_t[:], la_t[:], AF.Copy,
                                         accum_out=lac[sc][:, k:k + 1])

            # epilogue: per seg-chunk log_softmax over K + logsumexp over K
            lp_total = epi_p.tile([128, SC], F32, tag="lp_total")
            for sc in range(SC):
                radj = epi_p.tile([128, K], F32, tag="radj")
                nc.vector.tensor_sub(radj[:], a1[sc][:], a2[sc][:])
                cinv = epi_p.tile([128, 1], F32, tag="cinv")
                nc.vector.reciprocal(cinv[:], cnt_t[:, sc:sc + 1])
                rla = epi_p.tile([128, K], F32, tag="rla")
                nc.vector.tensor_scalar_mul(rla[:], lac[sc][:], cinv[:])

                m1n = epi_p.tile([128, 1], F32, tag="m1n")
                nc.vector.tensor_reduce(out=m1n[:], in_=rla[:], axis=AX.X,
                                        op=ALU.max, negate=True)
                e1 = epi_p.tile([128, K], F32, tag="e1")
                s1 = epi_p.tile([128, 1], F32, tag="s1")
                nc.scalar.activation(e1[:], rla[:], AF.Exp, bias=m1n[:],
                                     accum_out=s1[:])
                l1 = epi_p.tile([128, 1], F32, tag="l1")
                nc.scalar.activation(l1[:], s1[:], AF.Ln)
                b = epi_p.tile([128, 1], F32, tag="b")
                nc.vector.tensor_sub(b[:], m1n[:], l1[:])

                t1 = epi_p.tile([128, K], F32, tag="t1")
                nc.vector.tensor_sub(t1[:], rla[:], radj[:])
                z = epi_p.tile([128, K], F32, tag="z")
                nc.vector.tensor_scalar_add(z[:], t1[:], b[:])

                m2n = epi_p.tile([128, 1], F32, tag="m2n")
                nc.vector.tensor_reduce(out=m2n[:], in_=z[:], axis=AX.X,
                                        op=ALU.max, negate=True)
                e2 = epi_p.tile([128, K], F32, tag="e2")
                s2 = epi_p.tile([128, 1], F32, tag="s2")
                nc.scalar.activation(e2[:], z[:], AF.Exp, bias=m2n[:],
                                     accum_out=s2[:])
                l2 = epi_p.tile([128, 1], F32, tag="l2")
                nc.scalar.activation(l2[:], s2[:], AF.Ln)
                nc.vector.tensor_sub(lp_total[:, sc:sc + 1], l2[:], m2n[:])

            row = epi_p.tile([128, 1], F32, tag="row")
            nc.vector.tensor_reduce(out=row[:], in_=lp_total[:], axis=AX.X,
                                    op=ALU.add)
            ps_t = ps_p.tile([1, 1], F32, tag="ps")
            nc.tensor.matmul(ps_t[:], ones_t[:], row[:],
                             start=True, stop=True)
            res_t = epi_p.tile([1, 1], F32, tag="res")
            nc.vector.tensor_copy(res_t[:], ps_t[:])
            nc.sync.dma_start(out_ext[:], res_t[:])

    split_multi_waits(nc)
    return nc


def stage_inputs(label, log_theta, log_alpha, subgraph_idx):
    """Sort/pad/shard the inputs into the per-core staged layout.

    Returns (in_maps, L): in_maps[c] feeds core c.
    """
    label = np.asarray(label, np.float32)
    log_theta = np.ascontiguousarray(np.asarray(log_theta, np.float32))
    log_alpha = np.ascontiguousarray(np.asarray(log_alpha, np.float32))
    idx = np.asarray(subgraph_idx).astype(np.int64)

    counts = np.bincount(idx, minlength=S).astype(np.int64)
    L = int(counts.max())
    L = (L + 15) // 16 * 16  # align free dim

    order = np.argsort(idx, kind="stable").astype(np.int64)
    starts = np.zeros(S, np.int64)
    np.cumsum(counts[:-1], out=starts[1:])
    pos_in_seg = np.arange(E, dtype=np.int64) - starts[idx[order]]
    eidx = np.full((S, L), E, dtype=np.int64)
    eidx[idx[order], pos_in_seg] = order

    ltx = np.vstack([log_theta, np.full((1, K), LT_PAD, np.float32)])
    lax = np.vstack([log_alpha, np.zeros((1, K), np.float32)])
    lbx = np.concatenate([label, np.zeros(1, np.float32)])

    # [S, L, K] -> [cores, K, SC, 128, L]
    lt_g = ltx[eidx]          # [4096, L, 20]
    lt_g = lt_g.reshape(N_CORES, SC, 128, L, K).transpose(0, 4, 1, 2, 3)
    lt_g = np.ascontiguousarray(lt_g)
    la_g = lax[eidx]
    la_g = la_g.reshape(N_CORES, SC, 128, L, K).transpose(0, 4, 1, 2, 3)
    la_g = np.ascontiguousarray(la_g)
    lb_g = np.ascontiguousarray(lbx[eidx].reshape(N_CORES, SC, 128, L))
    cnt_g = np.ascontiguousarray(
        counts.astype(np.float32).reshape(N_CORES, SC, 128).transpose(0, 2, 1))

    in_maps = [
        {"lt": lt_g[c], "la": la_g[c], "lb": lb_g[c], "cnt": cnt_g[c]}
        for c in range(N_CORES)
    ]
    return in_maps, L


def finish(partials):
    """Combine the 8 per-core partial sums into the scalar loss."""
    total = np.sum([np.float64(p) for p in partials])
    return np.float32(-total / E)


def kernel(label, log_theta, log_alpha, subgraph_idx):
    in_maps, L = stage_inputs(label, log_theta, log_alpha, subgraph_idx)
    nc = build_graph(L)
    res = run_bass_kernel_spmd(nc, in_maps, core_ids=list(range(N_CORES)))
    return finish([res.results[c]["out"][0, 0] for c in range(N_CORES)])
